# revision 5
# baseline (speedup 1.0000x reference)
"""AttnDecoderRNN Trainium2 kernel, v5.

B=128 data-parallel over 8 cores (BL=16/core). Per core the 16 batches form
two phase-shifted groups of 8 (A leads, B lags half a step): the ACT
engine's big tanh(U+q) for one group overlaps the other group's
softmax/ctx/gates/LSTM tail.

Heavy per-step math uses fp8 DoubleRow matmuls with the LARGE tensor as the
stationary operand (dual-row fp8 weight loads halve the PE time; M=128,
col 0 satisfies the dual-fp8 ISA restrictions):
  scoresT[t, b] = X8[h,t].T @ va8[h,1]       (X fp8 stationary, va moving)
  ctxT[h, b]    = enc8[t,h].T @ w8[t,1]      (enc fp8 stationary, w moving)
scoresT is transposed back (PE) to [b, t] for a dense softmax without max
subtraction; w is normalized and scaled by 64 before the ctx matmul (the
1/64 is removed when ctxT is quantized to fp8), so flat-attention weights
stay out of the fp8 subnormal range.

Gates: one [128,512] PSUM tile per group, 4 gates at column bands
{0,32,64,96}; i16-selection matmul adds the per-batch bias; Whh part in
bf16, ctx part in plain fp8; a single full-tile ACT tanh with a
per-partition scale (0.5, g gate 1.0) converts all 4 bands.  LSTM cell as 3
scalar_tensor_tensor ops with h,c stored as 2h,2c (0.5 folded into
Wa/Whh/Wp host-side); partition-base remaps via legal ACT/copy/out remaps.
"""

import numpy as np
import ml_dtypes
from contextlib import ExitStack

import concourse.bass as bass
import concourse.tile as tile
from concourse import bacc, mybir
from concourse.bass_types import DynSlice
from concourse.bass_utils import run_bass_kernel_spmd

F32 = mybir.dt.float32
BF16 = mybir.dt.bfloat16
F8 = mybir.dt.float8e4
AF = mybir.ActivationFunctionType
ALU = mybir.AluOpType
AX = mybir.AxisListType
PM = mybir.MatmulPerfMode

B, T, H, D = 128, 512, 512, 128
NCORES = 8
BL = B // NCORES   # 16
GB = BL // 2       # 8 per group
HC = H // 128      # 4
TC = T // 128      # 4
G4 = 4 * H         # 2048
VA_SCALE = 16.0
WN_SCALE = 64.0
HOOK_SLOTS = (1, 2, 4, 6)  # tail_start, tail_mid, tail_acts, tail_fin


def build(out_len: int, unroll: bool = False, bench_steps=None) -> bass.Bass:
    nc = bacc.Bacc(None, target_bir_lowering=False)

    encT = nc.dram_tensor("encT", [BL, HC, 128, T], BF16, kind="ExternalInput")
    uaT = nc.dram_tensor("uaT", [HC, 128, H], BF16, kind="ExternalInput")
    waT = nc.dram_tensor("waT", [HC, 128, H], BF16, kind="ExternalInput")
    whhT = nc.dram_tensor("whhT", [HC, 128, G4], BF16, kind="ExternalInput")
    wc8 = nc.dram_tensor("wc8", [HC, 128, G4], F8, kind="ExternalInput")
    wpT = nc.dram_tensor("wpT", [HC, 128, D], BF16, kind="ExternalInput")
    va8 = nc.dram_tensor("va8", [128, 2, 2], F8, kind="ExternalInput")
    enc8 = nc.dram_tensor("enc8", [BL, 2, 128, 2, H], F8, kind="ExternalInput")
    gcw = nc.dram_tensor("gcw", [BL, G4], BF16, kind="ExternalInput")
    bpw = nc.dram_tensor("bpw", [128, 1], F32, kind="ExternalInput")
    id8f = nc.dram_tensor("id8f", [8, 8], F32, kind="ExternalInput")
    id8b = nc.dram_tensor("id8b", [8, 8], BF16, kind="ExternalInput")
    id128b = nc.dram_tensor("id128b", [128, 128], BF16, kind="ExternalInput")
    i16b = nc.dram_tensor("i16b", [16, 16], BF16, kind="ExternalInput")
    gscw = nc.dram_tensor("gscw", [128, 1], F32, kind="ExternalInput")
    yTA = nc.dram_tensor("yTA", [out_len, 128, GB], F32, kind="ExternalOutput")
    yTB = nc.dram_tensor("yTB", [out_len, 128, GB], F32, kind="ExternalOutput")

    with tile.TileContext(nc) as tc, ExitStack() as ctx:
        singles = ctx.enter_context(tc.tile_pool(name="singles", bufs=1))
        U_sb = singles.tile([128, BL, HC, T], BF16)        # 64KB/part
        enc8_sb = singles.tile([128, BL, 2, 2, H], F8)     # 32KB/part
        waT_sb = singles.tile([128, HC, H], BF16)
        whhT_sb = singles.tile([128, HC, G4], BF16)
        wc8_sb = singles.tile([128, HC, G4], F8)
        wpT_sb = singles.tile([128, HC, D], BF16)
        va8_sb = singles.tile([128, 2, 2], F8)
        gc_sb = singles.tile([BL, G4], BF16)
        bp_sb = singles.tile([128, 1], F32)
        id8f_sb = singles.tile([8, 8], F32)
        id8b_sb = singles.tile([8, 8], BF16)
        id128b_sb = singles.tile([128, 128], BF16)
        i16b_sb = singles.tile([16, 16], BF16)
        gsc_sb = singles.tile([128, 1], F32)
        hsT_sb = singles.tile([128, HC, BL], BF16)         # 2h, transposed
        zcol_sb = singles.tile([1, 128], BF16)
        cs2_t = [singles.tile([128, H], F32, name=f"cs2_{gg}")
                 for gg in range(2)]                       # 2c at rows 32-39
        tgr_t = [singles.tile([GB, H], F32, name=f"tgr{gg}") for gg in range(2)]

        nc.gpsimd.dma_start(out=waT_sb[:], in_=waT.rearrange("k p t -> p k t"))
        nc.gpsimd.dma_start(out=whhT_sb[:], in_=whhT.rearrange("k p t -> p k t"))
        nc.gpsimd.dma_start(out=wc8_sb[:], in_=wc8.rearrange("k p t -> p k t"))
        nc.gpsimd.dma_start(out=wpT_sb[:], in_=wpT.rearrange("k p t -> p k t"))
        nc.gpsimd.dma_start(out=va8_sb[:], in_=va8[:])
        nc.gpsimd.dma_start(out=gc_sb[:], in_=gcw[:])
        nc.gpsimd.dma_start(out=bp_sb[:], in_=bpw[:])
        nc.gpsimd.dma_start(out=id8f_sb[:], in_=id8f[:])
        nc.gpsimd.dma_start(out=id8b_sb[:], in_=id8b[:])
        nc.gpsimd.dma_start(out=id128b_sb[:], in_=id128b[:])
        nc.gpsimd.dma_start(out=i16b_sb[:], in_=i16b[:])
        nc.gpsimd.dma_start(out=gsc_sb[:], in_=gscw[:])
        for b in range(BL):
            nc.gpsimd.dma_start(out=enc8_sb[:, b],
                                in_=enc8[b].rearrange("g p j h -> p g j h"))
        nc.vector.memset(hsT_sb[:], 0)
        nc.vector.memset(zcol_sb[:], 0)
        nc.vector.memset(cs2_t[0][:], 0)
        nc.vector.memset(cs2_t[1][:], 0)

        # --- pre-loop: U[b] = Ua @ enc[b]^T ---
        with tc.tile_pool(name="preloop", bufs=2) as prepool, \
             tc.tile_pool(name="pre_ps", bufs=2, space="PSUM") as preps:
            uaT_sb = prepool.tile([128, HC, H], BF16, tag="uaw")
            nc.gpsimd.dma_start(out=uaT_sb[:], in_=uaT.rearrange("k p t -> p k t"))
            for b in range(BL):
                est = prepool.tile([128, HC, T], BF16, tag="est")
                nc.gpsimd.dma_start(out=est[:], in_=encT[b].rearrange("k p t -> p k t"))
                for mc in range(HC):
                    pu = preps.tile([128, T], F32, tag="pre")
                    for kc in range(HC):
                        nc.tensor.matmul(
                            pu[:], uaT_sb[:, kc, mc * 128:(mc + 1) * 128],
                            est[:, kc, :], start=(kc == 0), stop=(kc == HC - 1))
                    nc.vector.tensor_copy(U_sb[:, b, mc, :], pu[:])

        tc.strict_bb_all_engine_barrier()

        # PSUM pools: scoresT 2 + qp 2 + gates 2 + small 2 = 8 banks
        ps_st = ctx.enter_context(tc.tile_pool(name="ps_st", bufs=2, space="PSUM"))
        ps_q = ctx.enter_context(tc.tile_pool(name="ps_q", bufs=2, space="PSUM"))
        ps_g = ctx.enter_context(tc.tile_pool(name="ps_g", bufs=2, space="PSUM"))
        ps_sm = ctx.enter_context(tc.tile_pool(name="ps_sm", bufs=2, space="PSUM"))

        work = ctx.enter_context(tc.tile_pool(name="work", bufs=2))
        w1 = ctx.enter_context(tc.tile_pool(name="w1", bufs=1))
        xpre_p = ctx.enter_context(tc.tile_pool(name="xpre", bufs=3))
        x8_p = ctx.enter_context(tc.tile_pool(name="x8", bufs=3))

        esc_t = {g: w1.tile([GB, T], F32, tag=f"esc{g}", name=f"esc{g}")
                 for g in range(2)}
        wn_t = {g: w1.tile([GB, T], BF16, tag=f"wn{g}", name=f"wn{g}")
                for g in range(2)}
        sT_t = {g: w1.tile([128, TC, GB], BF16, tag=f"sT{g}", name=f"sT{g}")
                for g in range(2)}
        rcp_t = {g: w1.tile([GB, 1], F32, tag=f"rcp{g}", name=f"rcp{g}")
                 for g in range(2)}
        w8_t = {g: w1.tile([128, 2, 2, GB], F8, tag=f"w8{g}", name=f"w8{g}")
                for g in range(2)}
        c8_t = {g: w1.tile([128, HC, GB], F8, tag=f"c8{g}", name=f"c8{g}")
                for g in range(2)}
        gt_t = {g: w1.tile([128, T], F32, tag=f"gt{g}", name=f"gt{g}")
                for g in range(2)}
        t1_t = {g: w1.tile([128, H], F32, tag=f"t1{g}", name=f"t1{g}")
                for g in range(2)}
        t2_t = {g: w1.tile([128, H], F32, tag=f"t2{g}", name=f"t2{g}")
                for g in range(2)}
        tc_t = {g: w1.tile([128, H], F32, tag=f"tc{g}", name=f"tc{g}")
                for g in range(2)}
        h2_t = {g: w1.tile([GB, H], F32, tag=f"h2{g}", name=f"h2{g}")
                for g in range(2)}

        def bsl(g):
            return slice(g * GB, (g + 1) * GB)

        st_live = {}

        def head_q(g):
            qp = ps_q.tile([128, HC, GB], F32, tag="q", name=f"qp{g}")
            for mc in range(HC):
                for kc in range(HC):
                    nc.tensor.matmul(
                        qp[:, mc, :], waT_sb[:, kc, mc * 128:(mc + 1) * 128],
                        hsT_sb[:, kc, bsl(g)], start=(kc == 0), stop=(kc == HC - 1))
            return qp

        def head_x(g, qp, hooks=()):
            # per b: DVE pre-add (U+q), ACT merged tanh -> X fp8, then
            # scoresT[:, tc, b] = sum_h X8[h, t]^T va8[h] (DR-stationary-X)
            st_ps = ps_st.tile([128, TC, GB], F32, tag="st", name=f"st{g}")
            st_live[g] = st_ps
            hd = {}
            for k, v in dict(hooks).items():
                hd.setdefault(k, []).extend(v)
            for j in range(GB):
                b = g * GB + j
                xp = xpre_p.tile([128, HC, T], BF16, tag="xpre", name=f"xp{b}")
                for hc in range(HC):
                    nc.vector.tensor_scalar(
                        xp[:, hc, :], U_sb[:, b, hc, :],
                        qp[:, hc, j:j + 1], None, ALU.add)
                x8 = x8_p.tile([128, 2, 2, T], F8, tag="x8", name=f"x8{b}")
                nc.scalar.activation(
                    out=x8.rearrange("p g j t -> p (g j t)"),
                    in_=xp.rearrange("p k t -> p (k t)"),
                    func=AF.Tanh, bias=0.0, scale=1.0)
                for t_c in range(TC):
                    for g2 in range(2):
                        nc.tensor.matmul(
                            st_ps[:, t_c, j:j + 1],
                            x8[:, g2, :, t_c * 128:(t_c + 1) * 128],
                            va8_sb[:, :, g2:g2 + 1],
                            start=(g2 == 0), stop=(g2 == 1),
                            perf_mode=PM.DoubleRow, skip_group_check=True)
                for fn in hd.get(j, ()):
                    fn()
            # free the PSUM tile in-window (avoids loop-carried liveness)
            nc.vector.tensor_copy(sT_t[g][:], st_ps[:])

        def tail_start(g):
            # dense scores via PE transpose, softmax without max subtraction
            sdn = ps_sm.tile([GB, TC, 128], BF16, tag="sm", name=f"sdn{g}")
            for t_c in range(TC):
                nc.tensor.transpose(sdn[:, t_c, :], sT_t[g][:, t_c, :],
                                    id128b_sb[:])
            esc = esc_t[g]
            nc.scalar.activation(out=esc[:], in_=sdn.rearrange("b k t -> b (k t)"),
                                 func=AF.Exp, bias=0.0, scale=1.0 / VA_SCALE)
            nc.vector.tensor_reduce(rcp_t[g][:], esc[:], axis=AX.X, op=ALU.add)
            nc.vector.tensor_scalar(rcp_t[g][:], rcp_t[g][:], 1.0 / WN_SCALE,
                                    None, ALU.mult)
            nc.vector.reciprocal(rcp_t[g][:], rcp_t[g][:])
            nc.vector.tensor_scalar(wn_t[g][:], esc[:], rcp_t[g][:], None,
                                    ALU.mult)

        def tail_mms(g):
            # wT transposes -> w8 fp8; ctxT via DR-stationary-enc; gates
            wtp = ps_sm.tile([128, TC, GB], BF16, tag="sm", name=f"wtp{g}")
            for t_c in range(TC):
                nc.tensor.transpose(wtp[:, t_c, :],
                                    wn_t[g][:, t_c * 128:(t_c + 1) * 128],
                                    id8b_sb[:])
            nc.vector.tensor_copy(
                w8_t[g].rearrange("p a j b -> p (a j b)"),
                wtp.rearrange("p k b -> p (k b)"))
            ct_ps = ps_sm.tile([128, HC, GB], F32, tag="sm", name=f"ct{g}")
            for j in range(GB):
                b = g * GB + j
                for hc in range(HC):
                    for tcc in range(2):
                        nc.tensor.matmul(
                            ct_ps[:, hc, j:j + 1],
                            enc8_sb[:, b, tcc, :, hc * 128:(hc + 1) * 128],
                            w8_t[g][:, tcc, :, j:j + 1],
                            start=(tcc == 0), stop=(tcc == 1),
                            perf_mode=PM.DoubleRow, skip_group_check=True)
            nc.vector.tensor_scalar(
                c8_t[g].rearrange("p k b -> p (k b)"),
                ct_ps.rearrange("p k b -> p (k b)"), 1.0 / WN_SCALE,
                None, ALU.mult)
            gp = ps_g.tile([128, T], F32, tag="g", name=f"gp{g}")
            nc.tensor.matmul(gp[:], zcol_sb[:], gc_sb[0:1, 0:T], start=True,
                             stop=False, skip_group_check=True,
                             tile_position=(0, 0))
            for gi in range(4):
                col = 32 * gi
                gs = slice(gi * H, (gi + 1) * H)
                nc.tensor.matmul(gp[col:col + GB, :], i16b_sb[:, bsl(g)],
                                 gc_sb[:, gs], start=False, stop=False,
                                 skip_group_check=True, tile_position=(0, col))
                for kc in range(HC):
                    nc.tensor.matmul(gp[col:col + GB, :], hsT_sb[:, kc, bsl(g)],
                                     whhT_sb[:, kc, gs], start=False,
                                     stop=False, skip_group_check=True,
                                     tile_position=(0, col))
                for kc in range(HC):
                    nc.tensor.matmul(gp[col:col + GB, :], c8_t[g][:, kc, :],
                                     wc8_sb[:, kc, gs], start=False,
                                     stop=(kc == HC - 1),
                                     skip_group_check=True,
                                     tile_position=(0, col))
            return gp

        def tail_act_gt(g, gp):
            nc.scalar.activation(out=gt_t[g][:], in_=gp[:], func=AF.Tanh,
                                 bias=0.0, scale=gsc_sb[:])

        def tail_cell(g):
            # bands: i@0, f@32, g@64, o@96; cs2 at rows 32-39.
            gt = gt_t[g]
            cs = cs2_t[g][32:32 + GB, :]
            nc.vector.tensor_copy(tgr_t[g][:], gt[64:64 + GB, :])
            nc.vector.scalar_tensor_tensor(
                t1_t[g][32:32 + GB, :], gt[32:32 + GB, :], 1.0, cs,
                ALU.add, ALU.mult)
            nc.vector.scalar_tensor_tensor(
                t2_t[g][32:32 + GB, :], gt[0:GB, :], 1.0, tgr_t[g][:],
                ALU.add, ALU.mult)
            nc.vector.scalar_tensor_tensor(
                cs, t1_t[g][32:32 + GB, :], 0.5, t2_t[g][32:32 + GB, :],
                ALU.mult, ALU.add)

        def tail_act_tc(g):
            nc.scalar.activation(out=tc_t[g][96:96 + GB, :],
                                 in_=cs2_t[g][32:32 + GB, :],
                                 func=AF.Tanh, bias=0.0, scale=0.5)

        def tail_h2(g):
            gt = gt_t[g]
            nc.vector.scalar_tensor_tensor(h2_t[g][:], gt[96:96 + GB, :], 1.0,
                                           tc_t[g][96:96 + GB, :],
                                           ALU.add, ALU.mult)

        def tail_finish(g, ysl):
            htp = ps_sm.tile([128, HC, GB], F32, tag="sm", name=f"htp{g}")
            for hc in range(HC):
                nc.tensor.transpose(htp[:, hc, :],
                                    h2_t[g][:, hc * 128:(hc + 1) * 128],
                                    id8f_sb[:])
            nc.vector.tensor_copy(hsT_sb[:, :, bsl(g)], htp[:])
            yp = ps_sm.tile([128, GB], F32, tag="sm", name=f"yp{g}")
            for kc in range(HC):
                nc.tensor.matmul(yp[:], wpT_sb[:, kc, :], hsT_sb[:, kc, bsl(g)],
                                 start=(kc == 0), stop=(kc == HC - 1))
            ys = work.tile([128, GB], F32, tag=f"y{g}")
            nc.vector.tensor_scalar(ys[:], yp[:], bp_sb[:], None, ALU.add)
            yt = yTA if g == 0 else yTB
            nc.sync.dma_start(out=yt[ysl], in_=ys[:])

        def emit_iteration(iv, first):
            ysl_prev = (slice(0, 1) if bench_steps else DynSlice(iv - 1, 1))
            ysl_cur = (slice(0, 1) if bench_steps else DynSlice(iv, 1))
            gpx = [None, None]

            def hk_start(g):
                return lambda: tail_start(g)

            def hk_mid(g):
                def f():
                    gpx[g] = tail_mms(g)
                return f

            def hk_acts(g):
                def f():
                    tail_act_gt(g, gpx[g])
                    tail_cell(g)
                    tail_act_tc(g)
                    tail_h2(g)
                return f

            def hk_fin(g, ysl):
                return lambda: tail_finish(g, ysl)

            js, jm, ja, jf = HOOK_SLOTS
            qpA = head_q(0)
            hooksA = {}
            if not first:
                hooksA = {js: [hk_start(1)], jm: [hk_mid(1)],
                          ja: [hk_acts(1)], jf: [hk_fin(1, ysl_prev)]}
            head_x(0, qpA, tuple(hooksA.items()))
            qpB = head_q(1)
            hooksB = {js: [hk_start(0)], jm: [hk_mid(0)],
                      ja: [hk_acts(0)], jf: [hk_fin(0, ysl_cur)]}
            head_x(1, qpB, tuple(hooksB.items()))

        n = bench_steps or out_len
        emit_iteration(0, True)
        if unroll:
            for i in range(1, n):
                emit_iteration(i, False)
        else:
            with tc.For_i(1, n, 1, hint_engines=(mybir.EngineType.PE,)) as i:
                emit_iteration(i, False)
        # epilogue: B tail for the last step
        tail_start(1)
        gpB = tail_mms(1)
        tail_act_gt(1, gpB)
        tail_cell(1)
        tail_act_tc(1)
        tail_h2(1)
        tail_finish(1, slice(0, 1) if bench_steps else slice(n - 1, n))

    nc.finalize()
    return nc


_CACHE = {}


def _get_nc(out_len):
    if out_len not in _CACHE:
        _CACHE[out_len] = build(out_len)
    return _CACHE[out_len]


def make_inputs(encoder_outputs, latent_h, Wa, Ua, Va, W_ih, W_hh, b_ih, b_hh,
                Wp, bp):
    bf = ml_dtypes.bfloat16
    f8 = ml_dtypes.float8_e4m3
    enc = np.asarray(encoder_outputs, np.float32)
    Wa = np.asarray(Wa, np.float32)
    Ua = np.asarray(Ua, np.float32)
    Va = np.asarray(Va, np.float32)
    W_ih = np.asarray(W_ih, np.float32)
    latent = np.asarray(latent_h, np.float32)

    encT_a = np.ascontiguousarray(
        enc.transpose(0, 2, 1).reshape(B, HC, 128, T)).astype(bf)
    uaT_a = np.ascontiguousarray(Ua.T.reshape(HC, 128, H)).astype(bf)
    waT_a = np.ascontiguousarray((0.5 * Wa.T).reshape(HC, 128, H)).astype(bf)
    whhT_a = np.ascontiguousarray(
        (0.5 * np.asarray(W_hh, np.float32).T).reshape(HC, 128, G4)).astype(bf)
    WcT = np.ascontiguousarray(W_ih[:, H:].T)  # (H, 4H)
    wc8_a = np.ascontiguousarray(WcT.reshape(HC, 128, G4)).astype(f8)
    wpT_a = np.ascontiguousarray(
        (0.5 * np.asarray(Wp, np.float32).T).reshape(HC, 128, D)).astype(bf)
    va16 = VA_SCALE * Va[0]
    va8_a = np.ascontiguousarray(
        va16.reshape(2, 2, 128).transpose(2, 1, 0)).astype(f8)
    enc8_a = np.ascontiguousarray(
        enc.reshape(B, 2, 2, 128, H).transpose(0, 1, 3, 2, 4)).astype(f8)
    gc_a = (latent @ W_ih[:, :H].T + np.asarray(b_ih, np.float32)
            + np.asarray(b_hh, np.float32)).astype(bf)
    bp_a = np.asarray(bp, np.float32).reshape(128, 1)
    gsc_a = np.full((128, 1), 0.5, np.float32)
    gsc_a[64:64 + GB] = 1.0
    return dict(encT=encT_a, uaT=uaT_a, waT=waT_a, whhT=whhT_a, wc8=wc8_a,
                wpT=wpT_a, va8=va8_a, enc8=enc8_a, gcw=gc_a, bpw=bp_a,
                id8f=np.eye(8, dtype=np.float32),
                id8b=np.eye(8).astype(bf),
                id128b=np.eye(128).astype(bf),
                i16b=np.eye(16).astype(bf), gscw=gsc_a)


def kernel(encoder_outputs, latent_h, Wa, Ua, Va, W_ih, W_hh, b_ih, b_hh,
           Wp, bp, out_len):
    out_len = int(out_len)
    full = make_inputs(encoder_outputs, latent_h, Wa, Ua, Va, W_ih, W_hh,
                       b_ih, b_hh, Wp, bp)
    shard_keys = ("encT", "enc8", "gcw")
    nc = _get_nc(out_len)
    in_maps = []
    for c in range(NCORES):
        s = slice(c * BL, (c + 1) * BL)
        m = {k: (v[s] if k in shard_keys else v) for k, v in full.items()}
        in_maps.append(m)
    import os
    trace = bool(os.environ.get("KERNEL_TRACE"))
    res = run_bass_kernel_spmd(nc, in_maps, core_ids=list(range(NCORES)),
                               trace=trace)
    if res.exec_time_ns is not None:
        print(f"HW exec time: {res.exec_time_ns} ns", flush=True)
    outs = []
    for r in res.results:
        ya = r["yTA"].transpose(2, 0, 1)
        yb = r["yTB"].transpose(2, 0, 1)
        outs.append(np.concatenate([ya, yb], axis=0))
    return np.concatenate(outs, axis=0).astype(np.float32)
